# revision 24
# baseline (speedup 1.0000x reference)
"""Weighted-BCE loss kernel for Trainium2 (8 NeuronCores, SPMD data-parallel).

Reference math (torch-style BCELoss with class-balancing weights):
    n   = len(x), s = sum(gt)
    w0  = n / (2*(n-s)),  w1 = n / (2*s)
    L1  = max(log(x),     -100)
    L0  = max(log1p(-x),  -100)
    loss = mean( where(gt==0, w0, w1) * -(gt*L1 + (1-gt)*L0) )

Only ONE of log(x) / log(1-x) is needed per element (selected by gt), so
instead of two Ln passes we compute the selected operand in one shot:
    z = gt ? x : 1-x  =  1 - |x' - gt|,   x' = max(x, 2^-24)
(the clamp rides the op0 slot of the w-STT for free and guarantees
z >= 2^-24, so Ln never sees 0; vs the reference's -100 clamp this only
misvalues exact x==0 elements - ~1 in 16.7M, error ~5e-6 of the loss).

Global sums, all computed shard-locally (weights need only GLOBAL s):
    A  = sum(gt * Lz)        [DVE STT accum]  = sum_{gt=1} log x
    T  = sum(Lz)             [ACT accum, free on the Ln pass]
    W  = sum(x' - gt)        [free accum on the w-STT]
    Sx = sum(x)              [PE column-sum matmuls -> one PSUM bank]
    s  = Sx - W  (exact to ~1e-6 rel), so no separate sum(gt) pass.
    loss = -( A/(2s) + (T-A)/(2(n-s)) )

Measured engine rates (dtype-independent): DVE STT 1.08ns/col, ACT pass
0.91ns/col, DMA 420-429 GB/s per core - but ONLY with >=16KB descriptors
(4096 f32 cols/partition); 2048-col transfers drop to ~280 GB/s.  So DMA
granularity (4096-col chunks: x on the SP HWDGE queue, gt on the idle
Pool/GpSimd queue) is decoupled from compute granularity (2048-col
sub-tiles, with smaller first/last ones to cut ramp and drain):
    per 2048 sub-tile: DVE w-STT 2.4 + A-STT 2.4; ACT Abs 1.9 + Ln 1.9
    per 4096 chunk:    PE 8x[128,512] colsum matmuls into PSUM
    DMA pace:          4096-chunk x+gt = 4MB ~ 9.3us @ 429 GB/s
The A-STT is emitted one sub-tile late so DVE always has the next
w-STT queued ahead of the cross-engine Ln dependency; a dummy Ln in
the preamble pre-loads the natural_log act table (abs/ln/copy) so no
table swap lands mid-pipeline.  Host gathers the [128, 3*NT] accums +
the [1, 512] PSUM colsum from all 8 cores and finishes the (tiny)
all-reduce + scalar math in float64.
"""

import numpy as np
from contextlib import ExitStack

import concourse.bass as bass
import concourse.bacc as bacc
import concourse.mybir as mybir
import concourse.tile as tile
from concourse.alu_op_type import AluOpType
from concourse.bass_utils import run_bass_kernel_spmd

N_TOTAL = 16777216
N_CORES = 8
PER_CORE = N_TOTAL // N_CORES   # 2097152
P = 128
FD = PER_CORE // P              # 16384 free elements per partition
# DMA chunks (16KB/partition descriptors sustain peak HBM bandwidth; the
# first is small so compute starts ~2.5us after the queue opens)
CHUNKS = [1024, 2048, 4096, 4096, 4096, 1024]
assert sum(CHUNKS) == FD
# compute sub-tiles; each must lie inside a single DMA chunk
TILE_SIZES = [1024, 1024, 1024, 2048, 2048, 2048, 2048, 2048, 1024, 1024, 512, 512]
assert sum(TILE_SIZES) == FD
NT = len(TILE_SIZES)
MM = 512                        # moving free-dim chunk for PE colsums
X_CLAMP = 5.9604645e-08         # 2^-24: keeps z = 1-|x'-gt| >= 2^-24
LOG_CLAMP = -100.0

# Optional instrumentation knobs for a driver script (harness never sets them).
TRACE = False
LAST_RESULTS = None

_NC_CACHE = None


def _build():
    f32 = mybir.dt.float32
    i32 = mybir.dt.int32
    Ln = mybir.ActivationFunctionType.Ln
    Abs = mybir.ActivationFunctionType.Abs

    nc = bacc.Bacc("TRN2")
    x_in = nc.declare_dram_parameter("x", [P, FD], f32, isOutput=False)
    g_in = nc.declare_dram_parameter("gt", [P, FD], i32, isOutput=False)
    # packed accum output: columns [A | T | W], NT each
    out_all = nc.declare_dram_parameter("out_all", [P, 3 * NT], f32, isOutput=True)
    sum_x = nc.declare_dram_parameter("sum_x", [1, MM], f32, isOutput=True)

    n_mm = sum(t // MM for t in CHUNKS)

    with tile.TileContext(nc) as tc, ExitStack() as ctx:
        xp = ctx.enter_context(tc.tile_pool(name="xp", bufs=3))
        gp = ctx.enter_context(tc.tile_pool(name="gp", bufs=3))
        wp = ctx.enter_context(tc.tile_pool(name="wp", bufs=5))
        lp = ctx.enter_context(tc.tile_pool(name="lp", bufs=4))
        jp = ctx.enter_context(tc.tile_pool(name="jp", bufs=3))
        accp = ctx.enter_context(tc.tile_pool(name="accp", bufs=1))
        pp = ctx.enter_context(tc.psum_pool(name="pp", bufs=1))

        # one packed accum tile -> one output DMA
        acc_all = accp.tile([P, 3 * NT], f32)

        ones = accp.tile([P, 1], f32)
        nc.gpsimd.memset(ones[:], 1.0)

        # dummy Ln: forces the natural_log act-table (contains abs/ln/copy)
        # to load during the preamble instead of mid-pipeline
        warm = accp.tile([P, 1], f32)
        nc.scalar.activation(warm[:], ones[:], Ln)

        psum_t = pp.tile([1, MM], f32)

        def col(group, i):
            return acc_all[:, group * NT + i : group * NT + i + 1]

        def emit_A(i, lz, gt_t, gsl, tfd):
            junk_a = jp.tile([P, tfd], f32, tag="junk_a")
            nc.vector.scalar_tensor_tensor(
                junk_a[:], lz[:], LOG_CLAMP, gt_t[:, gsl],
                AluOpType.max, AluOpType.mult,
                accum_out=col(0, i),
            )

        # chunk iteration state
        chunk_iter = iter(CHUNKS)
        chunk_start = chunk_end = 0
        xt = gt_t = None
        mm_idx = 0
        pending_A = []  # (i, lz, gt_t, gt_slice, tfd): emitted 2 sub-tiles late

        off = 0
        for i, tfd in enumerate(TILE_SIZES):
            if off >= chunk_end:
                # start a new DMA chunk
                cw = next(chunk_iter)
                chunk_start, chunk_end = off, off + cw
                xt = xp.tile([P, cw], f32, tag="xt")
                gt_t = gp.tile([P, cw], i32, tag="gt")
                # two co-saturating HWDGE queues (SP + ACT, ~429 GB/s
                # combined): each chunk's x and gt stream CONCURRENTLY so
                # the pair completes together ~half a chunk earlier than a
                # single serial queue would deliver it
                nc.sync.dma_start(xt[:], x_in[:, chunk_start:chunk_end])
                nc.scalar.dma_start(gt_t[:], g_in[:, chunk_start:chunk_end])
                # Sx: accumulate column sums of x into one PSUM bank (idle PE)
                for c in range(0, cw, MM):
                    nc.tensor.matmul(
                        psum_t[:], ones[:], xt[:, c : c + MM],
                        start=(mm_idx == 0), stop=(mm_idx == n_mm - 1),
                    )
                    mm_idx += 1
            sl = slice(off - chunk_start, off - chunk_start + tfd)
            off += tfd

            # w = max(x, 2^-24) - gt in [-1, 1];  accum -> W
            wt = wp.tile([P, tfd], f32, tag="w")
            nc.vector.scalar_tensor_tensor(
                wt[:], xt[:, sl], X_CLAMP, gt_t[:, sl],
                AluOpType.max, AluOpType.subtract,
                accum_out=col(2, i),
            )
            # d = |w| in place (ACT), then Lz = Ln(1 - d), accum -> T
            nc.scalar.activation(wt[:], wt[:], Abs)
            lz = lp.tile([P, tfd], f32, tag="lz")
            nc.scalar.activation(
                lz[:], wt[:], Ln, bias=1.0, scale=-1.0,
                accum_out=col(1, i),
            )
            # A-STT deferred two sub-tiles (keeps independent DVE work
            # ahead of the cross-engine Ln dependency)
            pending_A.append((i, lz, gt_t, sl, tfd))
            if len(pending_A) > 2:
                emit_A(*pending_A.pop(0))

        for args in pending_A:
            emit_A(*args)

        nc.sync.dma_start(out_all[:, :], acc_all[:])
        sum_x_sb = accp.tile([1, MM], f32)
        nc.scalar.copy(sum_x_sb[:], psum_t[:])
        nc.sync.dma_start(sum_x[:, :], sum_x_sb[:])

    nc.compile()
    return nc


def get_nc():
    global _NC_CACHE
    if _NC_CACHE is None:
        _NC_CACHE = _build()
    return _NC_CACHE


def make_in_maps(x, gt):
    x = np.ascontiguousarray(np.asarray(x, dtype=np.float32).reshape(-1))
    gt = np.ascontiguousarray(np.asarray(gt, dtype=np.int32).reshape(-1))
    assert x.shape == (N_TOTAL,) and gt.shape == (N_TOTAL,)
    in_maps = []
    for c in range(N_CORES):
        sl = slice(c * PER_CORE, (c + 1) * PER_CORE)
        in_maps.append({
            "x": x[sl].reshape(P, FD),
            "gt": gt[sl].reshape(P, FD),
        })
    return in_maps


def combine(results):
    """All-reduce the per-core partial sums and finish the loss formula."""
    A = T = S = 0.0
    for r in results:
        o = r["out_all"].astype(np.float64)
        A += o[:, 0 * NT : 1 * NT].sum()
        T += o[:, 1 * NT : 2 * NT].sum()
        W = o[:, 2 * NT : 3 * NT].sum()
        Sx = r["sum_x"].astype(np.float64).sum()
        S += Sx - W                      # sum(gt) for this core
    n = float(N_TOTAL)
    result = -(A / (2.0 * S) + (T - A) / (2.0 * (n - S)))
    return np.array(result, dtype=np.float32)


def kernel(x, gt):
    global LAST_RESULTS
    nc = get_nc()
    in_maps = make_in_maps(x, gt)
    br = run_bass_kernel_spmd(nc, in_maps, list(range(N_CORES)))
    LAST_RESULTS = br
    return combine(br.results)


# revision 25
# speedup vs baseline: 1.0796x; 1.0796x over previous
"""Weighted-BCE loss kernel for Trainium2 (8 NeuronCores, SPMD data-parallel).

Reference math (torch-style BCELoss with class-balancing weights):
    n   = len(x), s = sum(gt)
    w0  = n / (2*(n-s)),  w1 = n / (2*s)
    L1  = max(log(x),     -100)
    L0  = max(log1p(-x),  -100)
    loss = mean( where(gt==0, w0, w1) * -(gt*L1 + (1-gt)*L0) )

Only ONE of log(x) / log(1-x) is needed per element (selected by gt), so
instead of two Ln passes we compute the selected operand in one shot:
    z = gt ? x : 1-x  =  1 - |x' - gt|,   x' = max(x, 2^-24)
(the clamp rides the op0 slot of the w-STT for free and guarantees
z >= 2^-24, so Ln never sees 0; vs the reference's -100 clamp this only
misvalues exact x==0 elements - ~1 in 16.7M, error ~5e-6 of the loss).

Global sums, all computed shard-locally (weights need only GLOBAL s):
    A  = sum(gt * Lz)        [DVE STT accum]  = sum_{gt=1} log x
    T  = sum(Lz)             [ACT accum, free on the Ln pass]
    W  = sum(x' - gt)        [free accum on the w-STT]
    Sx = sum(x)              [PE column-sum matmuls -> one PSUM bank]
    s  = Sx - W  (exact to ~1e-6 rel), so no separate sum(gt) pass.
    loss = -( A/(2s) + (T-A)/(2(n-s)) )

Dataflow (measured: DVE STT 1.08ns/col + ~0.15us/op, ACT pass 0.98ns/col,
DMA 429 GB/s with 16KB descriptors, less for smaller):
  - x and gt live in two fully-RESIDENT SBUF tensors (64KB/partition
    each).  All DMAs are pre-issued on the single SP HWDGE queue as
    interleaved x/gt chunk pairs (ramping chunk sizes: small first pair
    so compute starts ~11us, 4096-col chunks mid-stream for peak BW).
    Nothing downstream can ever stall the queue - compute sub-tiles
    just wait on the covering chunk-completion semaphores.
  - per sub-tile: DVE w-STT writes w; ACT Abs (in place) then
    Ln(1 - d) (in place, accum T) turn it into Lz; DVE A-STT (deferred
    two sub-tiles to stay decoupled from the cross-engine chain)
    reads Lz + gt.  In-place activations mean one [128, tfd] working
    tile per sub-tile: SBUF = 128K resident + ~56K working.
  - PE accumulates [128,512] column-sum matmuls of x into one PSUM
    bank; a dummy Ln in the preamble pre-loads the natural_log act
    table (abs/ln/copy) so no table swap lands mid-pipeline.
Host gathers the [128, 3*NT] accums + [1, 512] PSUM colsum from all 8
cores and finishes the (tiny) all-reduce + scalar math in float64.
"""

import numpy as np
from contextlib import ExitStack

import concourse.bass as bass
import concourse.bacc as bacc
import concourse.mybir as mybir
import concourse.tile as tile
from concourse.alu_op_type import AluOpType
from concourse.bass_utils import run_bass_kernel_spmd

N_TOTAL = 16777216
N_CORES = 8
PER_CORE = N_TOTAL // N_CORES   # 2097152
P = 128
FD = PER_CORE // P              # 16384 free elements per partition
# DMA chunk schedule, issued as interleaved x/gt pairs on one queue
CHUNKS = [1024, 1024, 2048, 4096, 4096, 4096]
assert sum(CHUNKS) == FD
# compute sub-tiles; each must lie inside a single DMA chunk
TILE_SIZES = [1024, 1024, 2048, 2048, 2048, 2048, 2048, 2048, 1024, 512, 512]
assert sum(TILE_SIZES) == FD
NT = len(TILE_SIZES)
MM = 512                        # moving free-dim chunk for PE colsums
X_CLAMP = 5.9604645e-08         # 2^-24: keeps z = 1-|x'-gt| >= 2^-24
LOG_CLAMP = -100.0

# Optional instrumentation knobs for a driver script (harness never sets them).
TRACE = False
LAST_RESULTS = None

_NC_CACHE = None


def _build():
    f32 = mybir.dt.float32
    i32 = mybir.dt.int32
    Ln = mybir.ActivationFunctionType.Ln
    Abs = mybir.ActivationFunctionType.Abs

    nc = bacc.Bacc("TRN2")
    x_in = nc.declare_dram_parameter("x", [P, FD], f32, isOutput=False)
    g_in = nc.declare_dram_parameter("gt", [P, FD], i32, isOutput=False)
    # packed accum output: columns [A | T | W], NT each
    out_all = nc.declare_dram_parameter("out_all", [P, 3 * NT], f32, isOutput=True)
    sum_x = nc.declare_dram_parameter("sum_x", [1, MM], f32, isOutput=True)

    n_mm = FD // MM

    with tile.TileContext(nc) as tc, ExitStack() as ctx:
        resp = ctx.enter_context(tc.tile_pool(name="resp", bufs=1))
        wp = ctx.enter_context(tc.tile_pool(name="wp", bufs=5))
        jp = ctx.enter_context(tc.tile_pool(name="jp", bufs=3))
        accp = ctx.enter_context(tc.tile_pool(name="accp", bufs=1))
        pp = ctx.enter_context(tc.psum_pool(name="pp", bufs=1))

        # fully-resident input tensors
        x_sb = resp.tile([P, FD], f32)
        g_sb = resp.tile([P, FD], i32)

        # pre-issue every DMA on the single SP queue as x/gt pairs
        off = 0
        for cw in CHUNKS:
            cs, ce = off, off + cw
            off += cw
            nc.sync.dma_start(x_sb[:, cs:ce], x_in[:, cs:ce])
            nc.sync.dma_start(g_sb[:, cs:ce], g_in[:, cs:ce])

        # one packed accum tile -> one output DMA
        acc_all = accp.tile([P, 3 * NT], f32)

        ones = accp.tile([P, 1], f32)
        nc.gpsimd.memset(ones[:], 1.0)

        # dummy Ln: forces the natural_log act-table (contains abs/ln/copy)
        # to load during the preamble instead of mid-pipeline
        warm = accp.tile([P, 1], f32)
        nc.scalar.activation(warm[:], ones[:], Ln)

        psum_t = pp.tile([1, MM], f32)

        def col(group, i):
            return acc_all[:, group * NT + i : group * NT + i + 1]

        def emit_A(i, lz, gsl, tfd):
            junk_a = jp.tile([P, tfd], f32, tag="junk_a")
            nc.vector.scalar_tensor_tensor(
                junk_a[:], lz[:], LOG_CLAMP, g_sb[:, gsl],
                AluOpType.max, AluOpType.mult,
                accum_out=col(0, i),
            )

        pending_A = []  # (i, lz_tile, gt_slice, tfd): emitted 2 sub-tiles late
        mm_idx = 0
        off = 0
        for i, tfd in enumerate(TILE_SIZES):
            sl = slice(off, off + tfd)
            off += tfd

            # w = max(x, 2^-24) - gt in [-1, 1];  accum -> W
            wt = wp.tile([P, tfd], f32, tag="w")
            nc.vector.scalar_tensor_tensor(
                wt[:], x_sb[:, sl], X_CLAMP, g_sb[:, sl],
                AluOpType.max, AluOpType.subtract,
                accum_out=col(2, i),
            )
            # Sx: accumulate column sums of x into one PSUM bank (idle PE)
            for c in range(sl.start, sl.stop, MM):
                nc.tensor.matmul(
                    psum_t[:], ones[:], x_sb[:, c : c + MM],
                    start=(mm_idx == 0), stop=(mm_idx == n_mm - 1),
                )
                mm_idx += 1
            # in place on ACT: d = |w|, then Lz = Ln(1 - d), accum -> T
            nc.scalar.activation(wt[:], wt[:], Abs)
            nc.scalar.activation(
                wt[:], wt[:], Ln, bias=1.0, scale=-1.0,
                accum_out=col(1, i),
            )
            # A-STT deferred two sub-tiles (keeps independent DVE work
            # ahead of the cross-engine Ln dependency)
            pending_A.append((i, wt, sl, tfd))
            if len(pending_A) > 2:
                emit_A(*pending_A.pop(0))

        for args in pending_A:
            emit_A(*args)

        nc.sync.dma_start(out_all[:, :], acc_all[:])
        sum_x_sb = accp.tile([1, MM], f32)
        nc.scalar.copy(sum_x_sb[:], psum_t[:])
        nc.sync.dma_start(sum_x[:, :], sum_x_sb[:])

    nc.compile()
    return nc


def get_nc():
    global _NC_CACHE
    if _NC_CACHE is None:
        _NC_CACHE = _build()
    return _NC_CACHE


def make_in_maps(x, gt):
    x = np.ascontiguousarray(np.asarray(x, dtype=np.float32).reshape(-1))
    gt = np.ascontiguousarray(np.asarray(gt, dtype=np.int32).reshape(-1))
    assert x.shape == (N_TOTAL,) and gt.shape == (N_TOTAL,)
    in_maps = []
    for c in range(N_CORES):
        sl = slice(c * PER_CORE, (c + 1) * PER_CORE)
        in_maps.append({
            "x": x[sl].reshape(P, FD),
            "gt": gt[sl].reshape(P, FD),
        })
    return in_maps


def combine(results):
    """All-reduce the per-core partial sums and finish the loss formula."""
    A = T = S = 0.0
    for r in results:
        o = r["out_all"].astype(np.float64)
        A += o[:, 0 * NT : 1 * NT].sum()
        T += o[:, 1 * NT : 2 * NT].sum()
        W = o[:, 2 * NT : 3 * NT].sum()
        Sx = r["sum_x"].astype(np.float64).sum()
        S += Sx - W                      # sum(gt) for this core
    n = float(N_TOTAL)
    result = -(A / (2.0 * S) + (T - A) / (2.0 * (n - S)))
    return np.array(result, dtype=np.float32)


def kernel(x, gt):
    global LAST_RESULTS
    nc = get_nc()
    in_maps = make_in_maps(x, gt)
    br = run_bass_kernel_spmd(nc, in_maps, list(range(N_CORES)))
    LAST_RESULTS = br
    return combine(br.results)
